# revision 1
# baseline (speedup 1.0000x reference)
import sys

sys.path.insert(0, "/opt/trn_rl_repo")

import numpy as np
import ml_dtypes

import concourse.bacc as bacc
import concourse.tile as tile
from concourse import mybir
from concourse.bass_utils import run_bass_kernel_spmd

F32 = mybir.dt.float32
BF16 = mybir.dt.bfloat16
Exp = mybir.ActivationFunctionType.Exp

B, T, E = 2, 2048, 1024
H, D = 16, 64
NCORES = 8
HPC = 4
QC = HPC * D
P = 128

_PROG = None


def _build():
    nc = bacc.Bacc("TRN2", target_bir_lowering=False, debug=False)

    xt_d = nc.dram_tensor("xt", [E, T], BF16, kind="ExternalInput")
    wqk_d = nc.dram_tensor("wqk", [E, 2 * QC], BF16, kind="ExternalInput")
    wv_d = nc.dram_tensor("wv", [E, QC], BF16, kind="ExternalInput")
    wp_d = nc.dram_tensor("wp", [QC, E], BF16, kind="ExternalInput")
    mk_d = nc.dram_tensor("masks", [P, 4, 512], BF16, kind="ExternalInput")
    out_d = nc.dram_tensor("out", [T, E], BF16, kind="ExternalOutput")

    KC = E // P
    NT = T // P

    with tile.TileContext(nc) as tc:
        with (
            tc.tile_pool(name="persist", bufs=1) as persist,
            tc.tile_pool(name="inp", bufs=1) as inp,
            tc.tile_pool(name="pt", bufs=4) as ptp,
            tc.tile_pool(name="small", bufs=2) as small,
            tc.tile_pool(name="denp", bufs=2) as denp,
            tc.tile_pool(name="stage", bufs=3) as stg,
            tc.tile_pool(name="big", bufs=2, space="PSUM") as big,
            tc.tile_pool(name="po", bufs=1, space="PSUM") as pop,
        ):
            qk_sb = [persist.tile([P, T], BF16, name=f"qk{m}") for m in range(4)]
            at_sb = [persist.tile([P, T], BF16, name=f"at{c}") for c in range(2)]
            v_sb = [persist.tile([P, HPC, D + 1], BF16, name=f"v{t}") for t in range(NT)]
            mask_sb = persist.tile([P, 4, 512], BF16, name="masks")
            wp_sb = [persist.tile([P, E], BF16, name=f"wp{c}") for c in range(2)]

            xt_sb = [inp.tile([P, T], BF16, name=f"xt{c}") for c in range(KC)]
            wqk_sb = [inp.tile([P, 2 * QC], BF16, name=f"wqk{c}") for c in range(KC)]
            wv_sb = [inp.tile([P, QC], BF16, name=f"wv{c}") for c in range(KC)]
            for c in range(KC):
                nc.sync.dma_start(out=wv_sb[c], in_=wv_d[c * P : (c + 1) * P, :])
                nc.sync.dma_start(
                    out=xt_sb[c][:, 0:1024], in_=xt_d[c * P : (c + 1) * P, 0:1024]
                )
            for c in range(KC):
                nc.sync.dma_start(out=wqk_sb[c], in_=wqk_d[c * P : (c + 1) * P, :])
            nc.sync.dma_start(out=mask_sb, in_=mk_d[:])
            for c in range(KC):
                nc.sync.dma_start(
                    out=xt_sb[c][:, 1024:2048],
                    in_=xt_d[c * P : (c + 1) * P, 1024:2048],
                )
            for c in range(2):
                nc.sync.dma_start(out=wp_sb[c], in_=wp_d[c * P : (c + 1) * P, :])

            ones4 = persist.tile([P, HPC, 1], F32, name="ones4")
            nc.vector.memset(ones4, 1.0)
            for t in range(NT):
                nc.vector.tensor_copy(v_sb[t][:, :, D : D + 1], ones4)

            def v_group(t, eng):
                ps = big.tile([P, 2, 512], F32, name="ps")
                for c in range(KC):
                    nc.tensor.matmul(
                        ps[:, 0, :QC],
                        lhsT=xt_sb[c][:, t * P : (t + 1) * P],
                        rhs=wv_sb[c],
                        start=(c == 0),
                        stop=(c == KC - 1),
                    )
                cp = nc.scalar.copy if eng == "s" else nc.vector.tensor_copy
                cp(
                    v_sb[t][:, :, 0:D],
                    ps[:, 0, :QC].rearrange("p (h d) -> p h d", h=HPC),
                )

            def qk_group(m, u, eng):
                ps = big.tile([P, 2, 512], F32, name="ps")
                for c in range(KC):
                    for nl in range(2):
                        nc.tensor.matmul(
                            ps[:, nl, :],
                            lhsT=wqk_sb[c][:, m * P : (m + 1) * P],
                            rhs=xt_sb[c][:, (2 * u + nl) * 512 : (2 * u + nl + 1) * 512],
                            start=(c == 0),
                            stop=(c == KC - 1),
                        )
                cp = nc.scalar.copy if eng == "s" else nc.vector.tensor_copy
                cp(
                    qk_sb[m][:, u * 1024 : (u + 1) * 1024],
                    ps.rearrange("p a b -> p (a b)"),
                )

            def out_tile(t, eng):
                ps = big.tile([P, 2, 512], F32, name="ps")
                for c in range(2):
                    for nl in range(2):
                        nc.tensor.matmul(
                            ps[:, nl, :],
                            lhsT=at_sb[c][:, t * P : (t + 1) * P],
                            rhs=wp_sb[c][:, nl * 512 : (nl + 1) * 512],
                            start=(c == 0),
                            stop=(c == 1),
                        )
                st = stg.tile([P, 1024], BF16, name="st")
                cp = nc.scalar.copy if eng == "s" else nc.vector.tensor_copy
                cp(st, ps.rearrange("p a b -> p (a b)"))
                nc.sync.dma_start(out=out_d[t * P : (t + 1) * P, :], in_=st)

            def attn_unit(po, p, fillers=()):
                Qh = qk_sb[po]
                Kh = qk_sb[2 + po]
                qbase = p * 1024
                psot = pop.tile([D + 1, 2, 2, 512], F32, name="pso")
                pso = [psot[:, 0], psot[:, 1]]
                nk = 8 * p + 8
                fills = {s: f for s, f in fillers}
                steps = []
                for i in range(nk):
                    for jj in range(2):
                        m = i - 8 * p - 4 * jj
                        if m > 3:
                            continue
                        steps.append((i, jj, max(0, 128 * m), m))
                jlast = {jj: max(i for i, j, _, _ in steps if j == jj) for jj in range(2)}

                def emit_pv(i, jj, w, pt):
                    for hh in range(2):
                        nc.tensor.matmul(
                            pso[hh][:, jj, w:512],
                            lhsT=v_sb[i][:, 2 * po + hh, :],
                            rhs=pt[:, hh, w:512],
                            start=(i == 0),
                            stop=(i == jlast[jj]),
                        )

                pend = None
                for s, (i, jj, w, m) in enumerate(steps):
                    ps = big.tile([P, 2, 512], F32, name="ps")
                    pt = ptp.tile([P, 2, 512], BF16, name="pt")
                    q0 = qbase + jj * 512
                    for hh in range(2):
                        nc.tensor.matmul(
                            ps[:, hh, w:512],
                            lhsT=Kh[64 * hh : 64 * hh + 64, i * P : (i + 1) * P],
                            rhs=Qh[64 * hh : 64 * hh + 64, q0 + w : q0 + 512],
                            start=True,
                            stop=True,
                        )
                    f = fills.get(s)
                    if f is not None:
                        f()
                    if pend is not None:
                        emit_pv(*pend)
                    nc.scalar.activation(pt[:, :, w:512], ps[:, :, w:512], Exp, scale=0.125)
                    if m >= 0:
                        for hh in range(2):
                            nc.vector.tensor_mul(
                                pt[:, hh, w:512],
                                pt[:, hh, w:512],
                                mask_sb[:, m, w:512],
                            )
                    pend = (i, jj, w, pt)
                emit_pv(*pend)
                dens = []
                for hh in range(2):
                    den = denp.tile([1, 2, 512], F32, name="den")
                    nc.scalar.copy(den, pso[hh][D : D + 1, :, :])
                    dens.append(den)
                for jj in range(2):
                    for hh in range(2):
                        rec = small.tile([1, 512], F32, name="rec")
                        rb = small.tile([64, 512], F32, name="rb")
                        nc.vector.reciprocal_approx_fast(out=rec, in_=dens[hh][:, jj, :])
                        nc.gpsimd.partition_broadcast(rb, rec)
                        nc.vector.tensor_mul(
                            at_sb[po][
                                64 * hh : 64 * hh + 64,
                                qbase + jj * 512 : qbase + (jj + 1) * 512,
                            ],
                            pso[hh][0:D, jj, :],
                            rb,
                        )

            for t in range(8):
                v_group(t, "s")
            qk_group(0, 0, "s")
            qk_group(2, 0, "s")
            attn_unit(
                0, 0,
                [(0, lambda: qk_group(1, 0, "v")), (1, lambda: qk_group(3, 0, "v"))]
                + [(6 + j, lambda t=t: v_group(t, "v")) for j, t in enumerate(range(8, 12))],
            )
            attn_unit(
                1, 0,
                [(0, lambda: qk_group(0, 1, "v")), (1, lambda: qk_group(2, 1, "v"))]
                + [(5 + j, lambda t=t: v_group(t, "v")) for j, t in enumerate(range(12, 16))],
            )
            attn_unit(
                0, 1,
                [(0, lambda: qk_group(1, 1, "v")), (1, lambda: qk_group(3, 1, "v"))]
                + [(8 + 4 * j, lambda t=t: out_tile(t, "v")) for j, t in enumerate(range(0, 4))],
            )
            attn_unit(
                1, 1,
                [(0, lambda: out_tile(4, "v")), (1, lambda: out_tile(5, "v")),
                 (12, lambda: out_tile(6, "v")), (22, lambda: out_tile(7, "v"))],
            )
            for t in range(8, NT):
                out_tile(t, "s" if t % 2 else "v")

    nc.compile()
    return nc


def _get_prog():
    global _PROG
    if _PROG is None:
        _PROG = _build()
    return _PROG


def _masks_np():
    kk = np.arange(P)[:, None]
    qq = np.arange(512)[None, :]
    return np.stack(
        [((128 * m + kk) <= qq) for m in range(4)], axis=1
    ).astype(ml_dtypes.bfloat16)


def _bf(a):
    return np.ascontiguousarray(a).astype(ml_dtypes.bfloat16)


def _shard(x, w_qkv, w_proj):
    masks = _masks_np()
    in_maps = []
    for core in range(NCORES):
        b, g = core // HPC, core % HPC
        c0 = g * QC
        in_maps.append(
            {
                "xt": _bf(x[b].T),
                "wqk": _bf(
                    np.concatenate(
                        [w_qkv[:, c0 : c0 + QC], w_qkv[:, E + c0 : E + c0 + QC]],
                        axis=1,
                    )
                ),
                "wv": _bf(w_qkv[:, 2 * E + c0 : 2 * E + c0 + QC]),
                "wp": _bf(w_proj[c0 : c0 + QC, :]),
                "masks": masks,
            }
        )
    return in_maps


def _run(inputs, **kwargs):
    x = np.asarray(inputs["x"], dtype=np.float32)
    w_qkv = np.asarray(inputs["w_qkv"], dtype=np.float32)
    w_proj = np.asarray(inputs["w_proj"], dtype=np.float32)
    b_proj = np.asarray(inputs["b_proj"], dtype=np.float32)

    nc = _get_prog()
    in_maps = _shard(x, w_qkv, w_proj)
    res = run_bass_kernel_spmd(nc, in_maps, core_ids=list(range(NCORES)), **kwargs)

    out = np.zeros((B, T, E), dtype=np.float32)
    for core in range(NCORES):
        out[core // HPC] += np.asarray(res.results[core]["out"], dtype=np.float32)
    out += b_proj[None, None, :]
    return out, res


def kernel(**inputs):
    out, _ = _run(inputs)
    return out



# revision 10
# speedup vs baseline: 1.0249x; 1.0249x over previous
import sys

sys.path.insert(0, "/opt/trn_rl_repo")

import numpy as np
import ml_dtypes

import concourse.bacc as bacc
import concourse.tile as tile
from concourse import mybir
from concourse.bass_utils import run_bass_kernel_spmd

F32 = mybir.dt.float32
BF16 = mybir.dt.bfloat16
Exp = mybir.ActivationFunctionType.Exp
BCAST_AP = False

B, T, E = 2, 2048, 1024
H, D = 16, 64
NCORES = 8
HPC = 4
QC = HPC * D
P = 128
KC = E // P
NT = T // P

_PROG = None


def _build():
    nc = bacc.Bacc("TRN2", target_bir_lowering=False, debug=False)

    xt_d = nc.dram_tensor("xt", [E, T], BF16, kind="ExternalInput")
    wqk_d = nc.dram_tensor("wqk", [E, 2 * QC], BF16, kind="ExternalInput")
    wv_d = nc.dram_tensor("wv", [E, QC], BF16, kind="ExternalInput")
    wp_d = nc.dram_tensor("wp", [QC, E], BF16, kind="ExternalInput")
    mk_d = nc.dram_tensor("masks", [P, 4, 512], BF16, kind="ExternalInput")
    out_d = nc.dram_tensor("out", [T, E], BF16, kind="ExternalOutput")

    with tile.TileContext(nc) as tc:
        with (
            tc.tile_pool(name="persist", bufs=1) as persist,
            tc.tile_pool(name="pt", bufs=3) as ptp,
            tc.tile_pool(name="small", bufs=2) as small,
            tc.tile_pool(name="stage", bufs=3) as stg,
            tc.tile_pool(name="spsum", bufs=1, space="PSUM") as spool,
            tc.tile_pool(name="ppsum", bufs=2, space="PSUM") as ppool,
            tc.tile_pool(name="fpsum", bufs=2, space="PSUM") as fpool,
        ):
            qk_sb = [persist.tile([P, T], BF16, name=f"qk{m}") for m in range(4)]
            at_sb = [persist.tile([P, T], BF16, name=f"at{c}") for c in range(2)]
            v_sb = [persist.tile([P, HPC, D + 1], BF16, name=f"v{t}") for t in range(NT)]
            mask_sb = persist.tile([P, 4, 512], BF16, name="masks")
            wp_sb = [persist.tile([P, E], BF16, name=f"wp{c}") for c in range(2)]
            xt_sb = [persist.tile([P, T], BF16, name=f"xt{c}") for c in range(KC)]
            wqk_sb = [persist.tile([P, 2 * QC], BF16, name=f"wqk{c}") for c in range(KC)]
            wv_sb = [persist.tile([P, QC], BF16, name=f"wv{c}") for c in range(KC)]

            for c in range(KC):
                nc.sync.dma_start(
                    out=wqk_sb[c][:, 0:256], in_=wqk_d[c * P : (c + 1) * P, 0:256]
                )
            for c in range(KC):
                nc.sync.dma_start(
                    out=xt_sb[c][:, 0:512], in_=xt_d[c * P : (c + 1) * P, 0:512]
                )
            for c in range(KC):
                nc.sync.dma_start(out=wv_sb[c], in_=wv_d[c * P : (c + 1) * P, :])
            nc.sync.dma_start(out=mask_sb, in_=mk_d[:])
            for c in range(KC):
                nc.sync.dma_start(
                    out=xt_sb[c][:, 512:1024], in_=xt_d[c * P : (c + 1) * P, 512:1024]
                )
            for c in range(KC):
                nc.sync.dma_start(
                    out=wqk_sb[c][:, 256:512], in_=wqk_d[c * P : (c + 1) * P, 256:512]
                )
            for tb in range(2, 4):
                for c in range(KC):
                    nc.sync.dma_start(
                        out=xt_sb[c][:, tb * 512 : (tb + 1) * 512],
                        in_=xt_d[c * P : (c + 1) * P, tb * 512 : (tb + 1) * 512],
                    )
            for c in range(2):
                nc.sync.dma_start(out=wp_sb[c], in_=wp_d[c * P : (c + 1) * P, :])

            ones4 = persist.tile([P, HPC, 1], F32, name="ones4")
            nc.vector.memset(ones4, 1.0)
            for t in range(NT):
                nc.vector.tensor_copy(v_sb[t][:, :, D : D + 1], ones4)

            cp_i = [0]

            def copy_out(dst, src):
                if cp_i[0] & 1:
                    nc.scalar.copy(dst, src)
                else:
                    nc.vector.tensor_copy(dst, src)
                cp_i[0] += 1

            st_tiles = {}

            def qk_chain(mb, u, nl):
                fp = fpool.tile([P, 512], F32, name="fq", tag="fp")
                for c in range(KC):
                    def mm(c=c, fp=fp):
                        nc.tensor.matmul(
                            fp,
                            lhsT=wqk_sb[c][:, mb * P : (mb + 1) * P],
                            rhs=xt_sb[c][:, u * 1024 + nl * 512 : u * 1024 + (nl + 1) * 512],
                            start=(c == 0),
                            stop=(c == KC - 1),
                        )
                    yield 512, mm
                nc.vector.tensor_copy(
                    qk_sb[mb][:, u * 1024 + nl * 512 : u * 1024 + (nl + 1) * 512], fp
                )

            def v_chain(t):
                fp = fpool.tile([P, 512], F32, name="fv", tag="fp")
                for c in range(KC):
                    def mm(c=c, fp=fp):
                        nc.tensor.matmul(
                            fp[:, 0:QC],
                            lhsT=xt_sb[c][:, t * P : (t + 1) * P],
                            rhs=wv_sb[c],
                            start=(c == 0),
                            stop=(c == KC - 1),
                        )
                    yield 256, mm
                nc.vector.tensor_copy(
                    v_sb[t][:, :, 0:D],
                    fp[:, 0:QC].rearrange("p (h d) -> p h d", h=HPC),
                )

            def out_chain(t, nl):
                fp = fpool.tile([P, 512], F32, name="fo", tag="fp")
                for c in range(2):
                    def mm(c=c, fp=fp):
                        nc.tensor.matmul(
                            fp,
                            lhsT=at_sb[c][:, t * P : (t + 1) * P],
                            rhs=wp_sb[c][:, nl * 512 : (nl + 1) * 512],
                            start=(c == 0),
                            stop=(c == 1),
                        )
                    yield 512, mm
                if t not in st_tiles:
                    st_tiles[t] = stg.tile([P, E], BF16, name=f"st{t}", tag="st")
                st = st_tiles[t]
                copy_out(st[:, nl * 512 : (nl + 1) * 512], fp)
                if nl == 1:
                    nc.sync.dma_start(out=out_d[t * P : (t + 1) * P, :], in_=st)

            specs = []
            specs += [(f"qk{mb}00", qk_chain(mb, 0, 0)) for mb in (0, 1)]
            specs += [(f"v{t}", v_chain(t)) for t in range(0, 4)]
            specs += [(f"qk{mb}01", qk_chain(mb, 0, 1)) for mb in (0, 1)]
            specs += [(f"v{t}", v_chain(t)) for t in range(4, 8)]
            specs += [(f"qk{mb}00", qk_chain(mb, 0, 0)) for mb in (2, 3)]
            specs += [(f"qk{mb}01", qk_chain(mb, 0, 1)) for mb in (2, 3)]
            specs += [("qk010", qk_chain(0, 1, 0)), ("qk110", qk_chain(1, 1, 0))]
            specs += [(f"v{t}", v_chain(t)) for t in range(8, 12)]
            specs += [("qk011", qk_chain(0, 1, 1)), ("qk111", qk_chain(1, 1, 1))]
            specs += [(f"out{t}.{nl}", out_chain(t, nl)) for t in range(0, 4) for nl in (0, 1)]
            specs += [("qk210", qk_chain(2, 1, 0)), ("qk310", qk_chain(3, 1, 0))]
            specs += [(f"v{t}", v_chain(t)) for t in range(12, 16)]
            specs += [(f"out{t}.{nl}", out_chain(t, nl)) for t in range(4, 8) for nl in (0, 1)]
            specs += [("qk211", qk_chain(2, 1, 1)), ("qk311", qk_chain(3, 1, 1))]
            specs += [(f"out{t}.{nl}", out_chain(t, nl)) for t in range(8, 12) for nl in (0, 1)]
            specs += [(f"out{t}.{nl}", out_chain(t, nl)) for t in range(12, 16) for nl in (0, 1)]

            names = [n for n, _ in specs]
            assert len(set(names)) == len(names)
            name_idx = {n: i for i, n in enumerate(names)}
            qpos = [0]
            done_names = set()

            def pull(budget_cols, limit=None):
                lim = len(specs) - 1 if limit is None else name_idx[limit]
                spent = 0
                while qpos[0] <= lim and spent < budget_cols:
                    name, g = specs[qpos[0]]
                    try:
                        cost, emit = next(g)
                    except StopIteration:
                        done_names.add(name)
                        qpos[0] += 1
                        continue
                    emit()
                    spent += cost
                return spent

            def drain_through(name):
                while name not in done_names:
                    if qpos[0] >= len(specs):
                        raise RuntimeError(f"drain_through({name}): queue empty")
                    pull(1 << 30, limit=name)

            def attn_chain(po, p, jj, deadlines=None, budget=1024, fence=None):
                deadlines = deadlines or {}
                Qh, Kh = qk_sb[2 * po], qk_sb[2 * po + 1]
                q0 = p * 1024 + jj * 512
                ilast = 8 * p + 4 * jj + 3
                pso = ppool.tile([D + 1, 2, 512], F32, name="pso", tag="pso")
                pend = None

                def emit_pv(i, w, pt):
                    for hh in range(2):
                        nc.tensor.matmul(
                            pso[:, hh, w:512],
                            lhsT=v_sb[i][:, 2 * po + hh, :],
                            rhs=pt[:, hh, w:512],
                            start=(i == 0),
                            stop=(i == ilast),
                        )

                for i in range(ilast + 1):
                    for nm in deadlines.get(i, ()):
                        drain_through(nm)
                    m = i - 8 * p - 4 * jj
                    w = max(0, 128 * m)
                    ps = spool.tile([P, 2, 512], F32, name="ps", tag="ps")
                    pt = ptp.tile([P, 2, 512], BF16, name="pt")
                    for hh in range(2):
                        nc.tensor.matmul(
                            ps[:, hh, w:512],
                            lhsT=Kh[64 * hh : 64 * hh + 64, i * P : (i + 1) * P],
                            rhs=Qh[64 * hh : 64 * hh + 64, q0 + w : q0 + 512],
                            start=True,
                            stop=True,
                        )
                    nc.scalar.activation(pt[:, :, w:512], ps[:, :, w:512], Exp, scale=0.125)
                    if pend is not None:
                        emit_pv(*pend)
                    pull(budget, limit=fence)
                    if m >= 0:
                        for hh in range(2):
                            nc.vector.tensor_mul(
                                pt[:, hh, w:512],
                                pt[:, hh, w:512],
                                mask_sb[:, m, w:512],
                            )
                    pend = (i, w, pt)
                emit_pv(*pend)

                den = small.tile([1, 2, 512], F32, name="den", tag="den")
                nc.scalar.copy(den, pso[D : D + 1, :, :])
                rec = small.tile([1, 2, 512], F32, name="rec")
                nc.vector.reciprocal_approx_fast(out=rec, in_=den)
                if BCAST_AP:
                    for hh in range(2):
                        nc.vector.tensor_mul(
                            at_sb[po][64 * hh : 64 * hh + 64, q0 : q0 + 512],
                            pso[0:D, hh, :],
                            rec[0:1, hh, :].partition_broadcast(D).squeeze(1),
                        )
                else:
                    rb = small.tile([D, 2, 512], F32, name="rb")
                    nc.gpsimd.partition_broadcast(rb, rec)
                    for hh in range(2):
                        nc.vector.tensor_mul(
                            at_sb[po][64 * hh : 64 * hh + 64, q0 : q0 + 512],
                            pso[0:D, hh, :],
                            rb[:, hh, :],
                        )

            drain_through("qk100")
            drain_through("v0")

            attn_chain(0, 0, 0, fence="qk111",
                       deadlines={1: ["v0"], 2: ["v1"], 3: ["v2", "v3"]})
            attn_chain(0, 0, 1, fence="qk111",
                       deadlines={0: ["qk001", "qk101"], 5: ["v4"], 6: ["v5"],
                                  7: ["v6", "v7"]})
            attn_chain(1, 0, 0, fence="qk111", deadlines={0: ["qk200", "qk300"]})
            attn_chain(1, 0, 1, fence="out3.1", deadlines={0: ["qk201", "qk301"]})
            attn_chain(0, 1, 0, fence="out7.1",
                       deadlines={0: ["qk010"], 8: ["qk110", "v8"], 9: ["v9"],
                                  10: ["v10"], 11: ["v11"]})
            attn_chain(0, 1, 1, fence="out7.1",
                       deadlines={0: ["qk011"], 12: ["qk111", "v12"], 13: ["v13"],
                                  14: ["v14"], 15: ["v15"]})
            attn_chain(1, 1, 0, fence="qk311", deadlines={0: ["qk210"], 8: ["qk310"]})
            attn_chain(1, 1, 1, fence="out11.1", deadlines={0: ["qk211"], 12: ["qk311"]})

            while qpos[0] < len(specs):
                pull(1 << 30)

    nc.compile()
    return nc


def _get_prog():
    global _PROG
    if _PROG is None:
        _PROG = _build()
    return _PROG


def _masks_np():
    kk = np.arange(P)[:, None]
    qq = np.arange(512)[None, :]
    return np.stack(
        [((128 * m + kk) <= qq) for m in range(4)], axis=1
    ).astype(ml_dtypes.bfloat16)


def _bf(a):
    return np.ascontiguousarray(a).astype(ml_dtypes.bfloat16)


def _shard(x, w_qkv, w_proj):
    masks = _masks_np()
    in_maps = []
    for core in range(NCORES):
        b, g = core // HPC, core % HPC
        c0 = g * QC
        wq = w_qkv[:, c0 : c0 + QC]
        wk = w_qkv[:, E + c0 : E + c0 + QC]
        wqk = np.concatenate(
            [wq[:, 0:128], wk[:, 0:128], wq[:, 128:256], wk[:, 128:256]], axis=1
        )
        in_maps.append(
            {
                "xt": _bf(x[b].T),
                "wqk": _bf(wqk),
                "wv": _bf(w_qkv[:, 2 * E + c0 : 2 * E + c0 + QC]),
                "wp": _bf(w_proj[c0 : c0 + QC, :]),
                "masks": masks,
            }
        )
    return in_maps


def _run(inputs, **kwargs):
    x = np.asarray(inputs["x"], dtype=np.float32)
    w_qkv = np.asarray(inputs["w_qkv"], dtype=np.float32)
    w_proj = np.asarray(inputs["w_proj"], dtype=np.float32)
    b_proj = np.asarray(inputs["b_proj"], dtype=np.float32)

    nc = _get_prog()
    in_maps = _shard(x, w_qkv, w_proj)
    res = run_bass_kernel_spmd(nc, in_maps, core_ids=list(range(NCORES)), **kwargs)

    out = np.zeros((B, T, E), dtype=np.float32)
    for core in range(NCORES):
        out[core // HPC] += np.asarray(res.results[core]["out"], dtype=np.float32)
    out += b_proj[None, None, :]
    return out, res


def kernel(**inputs):
    out, _ = _run(inputs)
    return out
